# revision 23
# baseline (speedup 1.0000x reference)
"""DeepSeekV2-style MLA attention forward on 8 Trainium2 NeuronCores.

Sharding: 2-way data-parallel over batch x 4-way tensor-parallel over heads
(4 heads per core). The shared low-rank projections (q_a / kv_a) are
replicated within each batch's TP group; the o_proj partial outputs are
summed on the host (TP unshard).

Layout convention on device: activations live transposed as [feature, token]
so that every matmul is out^T[f_out, t] = lhsT(W^T tile).T @ rhs(x^T tile),
with weights pre-transposed on the host. All matmul operands use the f32r
(reduced-precision fp32) PE path: 4x faster than fp32, ~1.4e-4 rel err.
"""
import math
import sys

import numpy as np

try:
    import concourse.bass as bass  # noqa: F401
except ImportError:  # pragma: no cover
    sys.path.insert(0, "/opt/trn_rl_repo")

import concourse.bass as bass
import concourse.tile as tile
from concourse import bacc, mybir
from concourse.bass_utils import run_bass_kernel_spmd

# ---- problem dims (hardcoded per contest contract) ----
B, S, HID = 2, 2048, 2048
NH = 16
DN, DR, DV = 128, 64, 128
QD = DN + DR                       # 192
QLR, KVLR = 1536, 512
EPS = 1e-6
ROPE_BASE = 10000.0
SCALE = 1.0 / math.sqrt(QD)

N_CORES = 8
TPG = 4                            # TP group size (cores per batch)
HPC = NH // TPG                    # heads per core = 4

F32 = mybir.dt.float32
F32R = mybir.dt.float32r
I32 = mybir.dt.int32

NKV = KVLR + DR                    # 576 kv_a rows
T_TILE = 512                       # token tile (free dim)
NT = S // T_TILE                   # 4 token tiles
KB = S // 128                      # 16 key tiles of 128

TWO_PI = 2.0 * math.pi
MAGIC = np.float32(1.5 * 2**23)    # round-to-nearest-int magic constant


def _cody_waite_consts():
    c1 = np.float32(np.float64(TWO_PI))
    c1 = np.frombuffer(
        (np.frombuffer(c1.tobytes(), np.uint32) & np.uint32(0xFFFFF000)).tobytes(),
        np.float32,
    )[0]
    r = np.float64(TWO_PI) - np.float64(c1)
    c2 = np.float32(r)
    c2 = np.frombuffer(
        (np.frombuffer(c2.tobytes(), np.uint32) & np.uint32(0xFFFFF000)).tobytes(),
        np.float32,
    )[0]
    c3 = np.float32(np.float64(TWO_PI) - np.float64(c1) - np.float64(c2))
    return float(c1), float(c2), float(c3)


CW1, CW2, CW3 = _cody_waite_consts()

_BUILD_CACHE = {}


def build_kernel(debug=False):
    key = bool(debug)
    if key in _BUILD_CACHE:
        return _BUILD_CACHE[key]

    nc = bacc.Bacc("TRN2", target_bir_lowering=False, debug=False,
                   num_devices=N_CORES)

    def din(name, shape, dt=F32R):
        return nc.dram_tensor(name, list(shape), dt, kind="ExternalInput").ap()

    # ---- per-core external inputs ----
    xT = din("xT", [HID, S])                       # hidden^T for this batch
    w_qaT = din("w_qaT", [HID, QLR])
    w_kvaT = din("w_kvaT", [HID, NKV])
    w_qbT = din("w_qbT", [QLR, HPC * QD])          # cols: nope h0..h3 | rope h0..h3
    w_kvb_nT = din("w_kvb_nT", [KVLR, HPC * DN])   # k_nope cols by head
    w_kvb_vT = din("w_kvb_vT", [KVLR, HPC * DV])   # v cols by head
    w_oT = din("w_oT", [HPC * DV, HID])
    b_qa = din("b_qa", [128, QLR // 128], F32)     # [128,12] col j = slice j
    b_kva = din("b_kva", [128, 5], F32)            # 576 padded to 640
    ln_qa = din("ln_qa", [128, QLR // 128], F32)
    ln_kva = din("ln_kva", [128, KVLR // 128], F32)
    pos = din("pos", [1, S], I32)
    inv_freq = din("inv_freq", [128, 1], F32)      # rope inv freqs, 4x repeated
    p128 = din("p128", [128, 128])                 # blockdiag(rotT, rotT) f32r
    ones_col = din("ones_col", [128, 1])           # f32r ones (rmsnorm lhsT)
    ones_c8 = din("ones_c8", [128, HPC * 2])       # f32r ones (v_aug columns)
    ident = din("ident", [128, 128])               # f32r identity (PE transpose)
    masks = din("masks", [4, 128, T_TILE])         # f32r causal diag masks

    out = nc.dram_tensor("out", [S, HID], F32, kind="ExternalOutput").ap()

    # ---- DRAM intermediates ----
    ikind = "ExternalOutput" if debug else "Internal"

    def dmid(name, shape, dt):
        return nc.dram_tensor(name, list(shape), dt, kind=ikind).ap()

    qa_val = dmid("qa_val", [QLR, S], F32)         # q_a pre-norm (with bias)
    kva_val = dmid("kva_val", [640, S], F32)       # kv_a pre-norm, 576 pad 640
    qT_nope = dmid("qT_nope", [HPC * DN, S], F32R)
    qT_rope = dmid("qT_rope", [HPC * DR, S], F32R)  # rope'd, by head
    k_nope = dmid("k_nope", [HPC * DN, S], F32R)
    k_rot = dmid("k_rot", [DR, S], F32R)           # shared rope'd key
    v_aug = dmid("v_aug", [KB, 128, HPC, DV + 2], F32R)
    attn_T = dmid("attn_T", [HPC * DV, S], F32R)
    if debug:
        dbg_sin = dmid("dbg_sin", [128, S], F32)
        dbg_cos = dmid("dbg_cos", [128, S], F32)
        dbg_rstd = dmid("dbg_rstd", [2, S], F32)

    NFO_QA = QLR // 128            # 12
    NFO_KV = KVLR // 128           # 4
    NHI = HID // 128               # 16

    with tile.TileContext(nc) as tc:
        # ---------- persistent small tiles ----------
        with tc.tile_pool(name="const", bufs=1) as constp:
            bqa_t = constp.tile([128, NFO_QA], F32)
            nc.sync.dma_start(bqa_t[:], b_qa[:])
            bkva_t = constp.tile([128, 5], F32)
            nc.sync.dma_start(bkva_t[:], b_kva[:])
            lnqa_t = constp.tile([128, NFO_QA], F32)
            nc.sync.dma_start(lnqa_t[:], ln_qa[:])
            lnkva_t = constp.tile([128, NFO_KV], F32)
            nc.sync.dma_start(lnkva_t[:], ln_kva[:])
            ones_t = constp.tile([128, 1], F32R)
            nc.sync.dma_start(ones_t[:], ones_col[:])
            ident_t = constp.tile([128, 128], F32R)
            nc.sync.dma_start(ident_t[:], ident[:])
            p128_t = constp.tile([128, 128], F32R)
            nc.sync.dma_start(p128_t[:], p128[:])
            ivf_t = constp.tile([128, 1], F32)
            nc.sync.dma_start(ivf_t[:], inv_freq[:])
            # rstd broadcast tiles (qa & kv), filled in phase A
            rstd_qa_b = constp.tile([128, NT, T_TILE], F32)
            rstd_kv_b = constp.tile([128, NT, T_TILE], F32)
            # cos/sin tables, filled in phase R
            cos_t = constp.tile([128, NT, T_TILE], F32)
            sin_t = constp.tile([128, NT, T_TILE], F32)

            # ---------- phase R: rope cos/sin tables ----------
            with nc.named_scope("rope_tables"), \
                 tc.tile_pool(name="ropep", bufs=1) as rp:
                pos_i = rp.tile([1, S], I32)
                nc.sync.dma_start(pos_i[:], pos[:])
                pos_f = rp.tile([1, S], F32)
                nc.vector.tensor_copy(pos_f[:], pos_i[:])
                pos_b = rp.tile([128, S], F32)
                nc.gpsimd.partition_broadcast(pos_b[:], pos_f[:])
                freqs = rp.tile([128, S], F32)
                nc.vector.tensor_scalar_mul(freqs[:], pos_b[:], ivf_t[:])
                kr = rp.tile([128, S], F32)
                nc.vector.tensor_scalar(kr[:], freqs[:], 1.0 / TWO_PI,
                                        float(MAGIC), mybir.AluOpType.mult,
                                        mybir.AluOpType.add)
                nc.vector.tensor_scalar_sub(kr[:], kr[:], float(MAGIC))
                red = rp.tile([128, S], F32)
                nc.vector.cody_waite_cascade(red[:], freqs[:], kr[:],
                                             CW1, CW2, CW3)
                nc.scalar.activation(sin_t.rearrange("p n t -> p (n t)"),
                                     red[:], mybir.ActivationFunctionType.Sin)
                redc = rp.tile([128, S], F32)
                nc.vector.add_range_wrap(redc[:], red[:], math.pi / 2.0,
                                         math.pi, TWO_PI)
                nc.scalar.activation(cos_t.rearrange("p n t -> p (n t)"),
                                     redc[:], mybir.ActivationFunctionType.Sin)
                if debug:
                    nc.sync.dma_start(dbg_sin[:],
                                      sin_t.rearrange("p n t -> p (n t)"))
                    nc.sync.dma_start(dbg_cos[:],
                                      cos_t.rearrange("p n t -> p (n t)"))

            # ---------- phase A: q_a / kv_a projections + rmsnorm stats ----
            with nc.named_scope("proj_a"):
                xT_r = xT.rearrange("(hi p) s -> p hi s", p=128)
                qa_val_r = qa_val.rearrange("(f p) s -> p f s", p=128)
                kva_val_r = kva_val.rearrange("(f p) s -> p f s", p=128)
                w_qaT_r = w_qaT.rearrange("(hi p) f -> p hi f", p=128)
                w_kvaT_r = w_kvaT.rearrange("(hi p) f -> p hi f", p=128)

                for half in range(2):
                    with tc.tile_pool(name="xa", bufs=1) as xap, \
                         tc.tile_pool(name="wa", bufs=3) as wap, \
                         tc.tile_pool(name="va", bufs=3) as vap, \
                         tc.tile_pool(name="pa", bufs=3, space="PSUM") as pap, \
                         tc.tile_pool(name="ssp", bufs=1, space="PSUM") as ssp:
                        xa = xap.tile([128, NHI, S // 2], F32R)
                        nc.sync.dma_start(
                            xa[:], xT_r[:, :, half * (S // 2):(half + 1) * (S // 2)])
                        ss_ps = {}
                        for proj in range(2):
                            nfo = NFO_QA if proj == 0 else 5
                            wsrc = w_qaT_r if proj == 0 else w_kvaT_r
                            vdst = qa_val_r if proj == 0 else kva_val_r
                            bias_t = bqa_t if proj == 0 else bkva_t
                            for fo in range(nfo):
                                m = 128 if not (proj == 1 and fo == 4) else 64
                                wt = wap.tile([128, NHI, 128], F32R, tag="wt")
                                nc.sync.dma_start(
                                    wt[:, :, :m],
                                    wsrc[:, :, fo * 128:fo * 128 + m])
                                for th in range(2):
                                    t = half * 2 + th
                                    ps = pap.tile([m, T_TILE], F32, tag="acc")
                                    for hi in range(NHI):
                                        nc.tensor.matmul(
                                            ps[:],
                                            wt[:, hi, :m],
                                            xa[:, hi, th * T_TILE:(th + 1) * T_TILE],
                                            start=(hi == 0), stop=(hi == NHI - 1))
                                    val = vap.tile([128, T_TILE], F32, tag="val")
                                    nc.vector.tensor_scalar_add(
                                        val[:m], ps[:], bias_t[:m, fo:fo + 1])
                                    nc.sync.dma_start(
                                        vdst[:m, fo, t * T_TILE:(t + 1) * T_TILE],
                                        val[:m])
                                    # rmsnorm stats (skip k_pe rows)
                                    if not (proj == 1 and fo == 4):
                                        sq = vap.tile([128, T_TILE], F32R, tag="sq")
                                        nc.vector.tensor_tensor(
                                            sq[:], val[:], val[:],
                                            mybir.AluOpType.mult)
                                        skey = (proj, t)
                                        if skey not in ss_ps:
                                            ss_ps[skey] = ssp.tile(
                                                [1, T_TILE], F32,
                                                name=f"ss_{proj}_{t}")
                                        nc.tensor.matmul(
                                            ss_ps[skey][:], ones_t[:], sq[:],
                                            start=(fo == 0),
                                            stop=(fo == nfo - 1 - (proj == 1)))
                        # rstd for this half's two token tiles
                        for proj in range(2):
                            d = QLR if proj == 0 else KVLR
                            dstb = rstd_qa_b if proj == 0 else rstd_kv_b
                            for th in range(2):
                                t = half * 2 + th
                                ms = vap.tile([1, T_TILE], F32, tag="ms")
                                nc.vector.tensor_scalar(
                                    ms[:], ss_ps[(proj, t)][:], 1.0 / d, EPS,
                                    mybir.AluOpType.mult, mybir.AluOpType.add)
                                std = vap.tile([1, T_TILE], F32, tag="std")
                                nc.scalar.activation(
                                    std[:], ms[:],
                                    mybir.ActivationFunctionType.Sqrt)
                                rstd = vap.tile([1, T_TILE], F32, tag="rstd")
                                nc.vector.reciprocal(rstd[:], std[:])
                                nc.gpsimd.partition_broadcast(
                                    dstb[:, t, :], rstd[:])
                                if debug:
                                    nc.sync.dma_start(
                                        dbg_rstd[proj:proj + 1,
                                                 t * T_TILE:(t + 1) * T_TILE],
                                        rstd[:])

            # ---------- phase B: q_b / kv_b / rope ----------
            qT_nope_r = qT_nope.rearrange("(f p) s -> p f s", p=128)
            qT_rope_r = qT_rope.rearrange("(f p) s -> p f s", p=64)
            k_nope_r = k_nope.rearrange("(f p) s -> p f s", p=128)
            w_qbT_r = w_qbT.rearrange("(fi p) f -> p fi f", p=128)
            w_kvb_nT_r = w_kvb_nT.rearrange("(fi p) f -> p fi f", p=128)
            w_kvb_vT_r = w_kvb_vT.rearrange("(fi p) f -> p fi f", p=128)

            NQB = HPC * QD // 128  # 6 output tiles (4 nope + 2 rope-pair)

            with nc.named_scope("proj_b"), \
                 tc.tile_pool(name="wb", bufs=3) as wbp, \
                 tc.tile_pool(name="rhb", bufs=2) as rhbp, \
                 tc.tile_pool(name="evb", bufs=2) as evbp, \
                 tc.tile_pool(name="pb", bufs=2, space="PSUM") as pbp:
                wv_t = wbp.tile([128, NFO_KV, HPC * DV], F32R, name="wv_t")
                nc.sync.dma_start(wv_t[:], w_kvb_vT_r[:])
                for t in range(NT):
                    tsl = slice(t * T_TILE, (t + 1) * T_TILE)
                    # load + normalize q_a rhs
                    qa_rhs = rhbp.tile([128, NFO_QA, T_TILE], F32R, tag="qarhs")
                    for f in range(NFO_QA):
                        valt = evbp.tile([128, T_TILE], F32, tag="ld")
                        nc.sync.dma_start(valt[:], qa_val_r[:, f, tsl])
                        nc.vector.scalar_tensor_tensor(
                            qa_rhs[:, f, :], valt[:], lnqa_t[:, f:f + 1],
                            rstd_qa_b[:, t, :],
                            mybir.AluOpType.mult, mybir.AluOpType.mult)
                    ckv_rhs = rhbp.tile([128, NFO_KV, T_TILE], F32R, tag="ckvrhs")
                    for f in range(NFO_KV):
                        valt = evbp.tile([128, T_TILE], F32, tag="ld")
                        nc.sync.dma_start(valt[:], kva_val_r[:, f, tsl])
                        nc.vector.scalar_tensor_tensor(
                            ckv_rhs[:, f, :], valt[:], lnkva_t[:, f:f + 1],
                            rstd_kv_b[:, t, :],
                            mybir.AluOpType.mult, mybir.AluOpType.mult)

                    # q_b: 4 nope tiles then 2 rope-pair tiles
                    for fo in range(NQB):
                        wt = wbp.tile([128, NFO_QA, 128], F32R, tag="wqb")
                        nc.sync.dma_start(
                            wt[:], w_qbT_r[:, :, fo * 128:(fo + 1) * 128])
                        ps = pbp.tile([128, T_TILE], F32, tag="qb")
                        for fi in range(NFO_QA):
                            nc.tensor.matmul(ps[:], wt[:, fi, :],
                                             qa_rhs[:, fi, :],
                                             start=(fi == 0),
                                             stop=(fi == NFO_QA - 1))
                        if fo < HPC:  # nope
                            ev = evbp.tile([128, T_TILE], F32R, tag="evr")
                            nc.scalar.activation(
                                ev[:], ps[:],
                                mybir.ActivationFunctionType.Copy)
                            nc.sync.dma_start(qT_nope_r[:, fo, tsl], ev[:])
                        else:  # rope pair: rows = heads (2j, 2j+1)
                            qpe = evbp.tile([128, T_TILE], F32R, tag="evr")
                            nc.scalar.activation(
                                qpe[:], ps[:],
                                mybir.ActivationFunctionType.Copy)
                            rps = pbp.tile([128, T_TILE], F32, tag="rot",
                                           bufs=1)
                            nc.tensor.matmul(rps[:], p128_t[:], qpe[:],
                                             start=True, stop=True)
                            tmp = evbp.tile([128, T_TILE], F32, tag="tmp")
                            nc.vector.tensor_tensor(tmp[:], cos_t[:, t, :],
                                                    qpe[:],
                                                    mybir.AluOpType.mult)
                            rot = evbp.tile([128, T_TILE], F32, tag="rot2")
                            nc.vector.tensor_tensor(rot[:], sin_t[:, t, :],
                                                    rps[:],
                                                    mybir.AluOpType.mult)
                            qro = evbp.tile([128, T_TILE], F32R, tag="evr2")
                            nc.vector.tensor_tensor(qro[:], tmp[:], rot[:],
                                                    mybir.AluOpType.add)
                            j = fo - HPC
                            nc.sync.dma_start(
                                qT_rope.rearrange("(f p) s -> p f s", p=128)[
                                    :, j, tsl],
                                qro[:])

                    # kv_b nope
                    for fo in range(HPC):
                        wt = wbp.tile([128, NFO_KV, 128], F32R, tag="wkn")
                        nc.sync.dma_start(
                            wt[:], w_kvb_nT_r[:, :, fo * 128:(fo + 1) * 128])
                        ps = pbp.tile([128, T_TILE], F32, tag="qb")
                        for fi in range(NFO_KV):
                            nc.tensor.matmul(ps[:], wt[:, fi, :],
                                             ckv_rhs[:, fi, :],
                                             start=(fi == 0),
                                             stop=(fi == NFO_KV - 1))
                        ev = evbp.tile([128, T_TILE], F32R, tag="evr")
                        nc.scalar.activation(ev[:], ps[:],
                                             mybir.ActivationFunctionType.Copy)
                        nc.sync.dma_start(k_nope_r[:, fo, tsl], ev[:])

                    # v (un-transposed): [tok, head*dv]
                    for ts in range(T_TILE // 128):
                        kb = t * 4 + ts
                        ps = pbp.tile([128, HPC * DV], F32, tag="vps", bufs=2)
                        for fi in range(NFO_KV):
                            nc.tensor.matmul(
                                ps[:],
                                ckv_rhs[:, fi, ts * 128:(ts + 1) * 128],
                                wv_t[:, fi, :],
                                start=(fi == 0), stop=(fi == NFO_KV - 1))
                        vt = evbp.tile([128, HPC, DV + 2], F32R, tag="vt")
                        nc.scalar.activation(
                            vt[:, :, 0:DV],
                            ps[:].rearrange("p (h d) -> p h d", h=HPC),
                            mybir.ActivationFunctionType.Copy)
                        nc.sync.dma_start(vt[:, :, DV:DV + 2],
                                          ones_c8.rearrange("p (h o) -> p h o",
                                                            o=2))
                        nc.sync.dma_start(v_aug[kb], vt[:])

                    # k_pe rope (shared across heads)
                    kpe = evbp.tile([64, T_TILE], F32R, tag="kpe")
                    valt = evbp.tile([64, T_TILE], F32, tag="ld64")
                    nc.sync.dma_start(valt[:],
                                      kva_val[KVLR:KVLR + DR, tsl])
                    nc.vector.tensor_copy(kpe[:], valt[:])
                    rps = pbp.tile([64, T_TILE], F32, tag="rotk", bufs=1)
                    nc.tensor.matmul(rps[:], p128_t[0:64, 0:64], kpe[:],
                                     start=True, stop=True)
                    tmp = evbp.tile([64, T_TILE], F32, tag="tmpk")
                    nc.vector.tensor_tensor(tmp[:], cos_t[0:64, t, :], kpe[:],
                                            mybir.AluOpType.mult)
                    rot = evbp.tile([64, T_TILE], F32, tag="rotk2")
                    nc.vector.tensor_tensor(rot[:], sin_t[0:64, t, :], rps[:],
                                            mybir.AluOpType.mult)
                    kro = evbp.tile([64, T_TILE], F32R, tag="kro")
                    nc.vector.tensor_tensor(kro[:], tmp[:], rot[:],
                                            mybir.AluOpType.add)
                    nc.sync.dma_start(k_rot[:, tsl], kro[:])

            # ---------- phase C: attention ----------
            attn_T_r = attn_T.rearrange("(f p) s -> p f s", p=128)
            with nc.named_scope("attn"), \
                 tc.tile_pool(name="kv", bufs=2) as kvp, \
                 tc.tile_pool(name="qrh", bufs=3) as qrhp, \
                 tc.tile_pool(name="pt", bufs=4) as ptp, \
                 tc.tile_pool(name="ao", bufs=4) as aop, \
                 tc.tile_pool(name="sps", bufs=3, space="PSUM") as spsp, \
                 tc.tile_pool(name="avs", bufs=1, space="PSUM") as avsp, \
                 tc.tile_pool(name="tps", bufs=1, space="PSUM") as tpsp:
                masks_t = kvp.tile([128, 4, T_TILE], F32R, name="masks_t",
                                   bufs=1)
                nc.sync.dma_start(masks_t[:], masks.rearrange("j p t -> p j t"))
                krot_sb = kvp.tile([64, S], F32R, name="krot_sb", bufs=1)
                nc.sync.dma_start(krot_sb[:], k_rot[:])
                for h in range(HPC):
                    kn_sb = kvp.tile([128, S], F32R, tag="kn")
                    nc.sync.dma_start(kn_sb[:], k_nope_r[:, h, :])
                    vh_sb = kvp.tile([128, KB, DV + 2], F32R, tag="vh")
                    nc.sync.dma_start(
                        vh_sb[:],
                        v_aug[:, :, h:h + 1, :].rearrange(
                            "kb p one d -> p kb (one d)"))
                    for qt in range(NT):
                        qsl = slice(qt * T_TILE, (qt + 1) * T_TILE)
                        qn_rhs = qrhp.tile([128, T_TILE], F32R, tag="qn")
                        nc.sync.dma_start(qn_rhs[:], qT_nope_r[:, h, qsl])
                        qr_rhs = qrhp.tile([64, T_TILE], F32R, tag="qr")
                        nc.sync.dma_start(qr_rhs[:], qT_rope_r[:, h, qsl])
                        av = [avsp.tile([128, DV + 2], F32, tag=f"av{i}",
                                        name=f"av{i}")
                              for i in range(4)]
                        nkb = 4 * qt + 4
                        for kb in range(nkb):
                            sps = spsp.tile([128, T_TILE], F32, tag="s")
                            nc.tensor.matmul(
                                sps[:], kn_sb[:, kb * 128:(kb + 1) * 128],
                                qn_rhs[:], start=True, stop=False)
                            nc.tensor.matmul(
                                sps[:], krot_sb[:, kb * 128:(kb + 1) * 128],
                                qr_rhs[:], start=False, stop=True)
                            pt = ptp.tile([128, T_TILE], F32R, tag="p")
                            nc.scalar.activation(
                                pt[:], sps[:],
                                mybir.ActivationFunctionType.Exp, scale=SCALE)
                            j = kb - 4 * qt
                            if j >= 0:
                                nc.vector.tensor_tensor(
                                    pt[:], pt[:], masks_t[:, j, :],
                                    mybir.AluOpType.mult)
                            for qs in range(4):
                                nc.tensor.matmul(
                                    av[qs][:],
                                    pt[:, qs * 128:(qs + 1) * 128],
                                    vh_sb[:, kb, :],
                                    start=(kb == 0), stop=(kb == nkb - 1))
                        for qs in range(4):
                            rec = aop.tile([128, 1], F32, tag="rec")
                            nc.vector.reciprocal(rec[:], av[qs][:, DV:DV + 1])
                            ao = aop.tile([128, DV], F32R, tag="ao")
                            nc.vector.tensor_scalar_mul(
                                ao[:], av[qs][:, 0:DV], rec[:])
                            tp = tpsp.tile([128, 128], F32R, tag="tp")
                            nc.tensor.transpose(tp[:], ao[:], ident_t[:])
                            aoT = aop.tile([128, 128], F32R, tag="aoT")
                            nc.scalar.activation(
                                aoT[:], tp[:],
                                mybir.ActivationFunctionType.Copy)
                            nc.sync.dma_start(
                                attn_T_r[:, h,
                                         qt * T_TILE + qs * 128:
                                         qt * T_TILE + (qs + 1) * 128],
                                aoT[:])

            # ---------- phase D: o_proj ----------
            w_oT_r = w_oT.rearrange("(fs p) hid -> p fs hid", p=128)
            with nc.named_scope("o_proj"), \
                 tc.tile_pool(name="wo", bufs=1) as wop, \
                 tc.tile_pool(name="at", bufs=3) as atp, \
                 tc.tile_pool(name="oe", bufs=3) as oep, \
                 tc.tile_pool(name="po", bufs=4, space="PSUM") as pop:
                wo_sb = wop.tile([128, HPC, HID], F32R)
                nc.sync.dma_start(wo_sb[:], w_oT_r[:])
                for ts in range(S // 128):
                    at_sb = atp.tile([128, HPC, 128], F32R, tag="at")
                    nc.sync.dma_start(
                        at_sb[:],
                        attn_T_r[:, :, ts * 128:(ts + 1) * 128])
                    for ho in range(HID // T_TILE):
                        ps = pop.tile([128, T_TILE], F32, tag="po")
                        for fs in range(HPC):
                            nc.tensor.matmul(
                                ps[:], at_sb[:, fs, :],
                                wo_sb[:, fs, ho * T_TILE:(ho + 1) * T_TILE],
                                start=(fs == 0), stop=(fs == HPC - 1))
                        oe = oep.tile([128, T_TILE], F32, tag="oe")
                        nc.scalar.activation(
                            oe[:], ps[:], mybir.ActivationFunctionType.Copy)
                        nc.sync.dma_start(
                            out[ts * 128:(ts + 1) * 128,
                                ho * T_TILE:(ho + 1) * T_TILE],
                            oe[:])

    nc.compile()
    _BUILD_CACHE[key] = nc
    return nc


def _host_consts():
    ivf = (1.0 / (ROPE_BASE ** (np.arange(0, DR, 2, dtype=np.float64) / DR)))
    ivf = ivf.astype(np.float32)                       # [32]
    inv_freq128 = np.tile(ivf, 4).reshape(128, 1)

    rot = np.zeros((DR, DR), np.float32)               # rot(x) = P @ x
    for d in range(32):
        rot[d, d + 32] = -1.0
        rot[d + 32, d] = 1.0
    rotT = rot.T
    p128 = np.zeros((128, 128), np.float32)
    p128[:64, :64] = rotT
    p128[64:, 64:] = rotT

    ident = np.eye(128, dtype=np.float32)

    kk = np.arange(128)[None, :, None]                 # [1,128,1]
    jj = np.arange(4)[:, None, None]                   # [4,1,1]
    qq = np.arange(T_TILE)[None, None, :]              # [1,1,512]
    masks = ((jj * 128 + kk) <= qq).astype(np.float32)  # [4,128,512]

    return inv_freq128, p128, ident, masks


LAST_RES = None


def kernel(_debug=False, **inputs):
    hidden_states = np.asarray(inputs["hidden_states"], np.float32)
    position_ids = np.asarray(inputs["position_ids"])
    W_qa = np.asarray(inputs["W_qa"], np.float32)
    b_qa = np.asarray(inputs["b_qa"], np.float32)
    w_qa_ln = np.asarray(inputs["w_qa_ln"], np.float32)
    W_qb = np.asarray(inputs["W_qb"], np.float32)
    W_kva = np.asarray(inputs["W_kva"], np.float32)
    b_kva = np.asarray(inputs["b_kva"], np.float32)
    w_kva_ln = np.asarray(inputs["w_kva_ln"], np.float32)
    W_kvb = np.asarray(inputs["W_kvb"], np.float32)
    W_o = np.asarray(inputs["W_o"], np.float32)

    nc = build_kernel(debug=_debug)

    inv_freq128, p128, ident, masks = _host_consts()

    w_qaT = np.ascontiguousarray(W_qa.T)
    w_kvaT = np.ascontiguousarray(W_kva.T)
    W_qb_h = W_qb.reshape(NH, QD, QLR)
    W_kvb_h = W_kvb.reshape(NH, DN + DV, KVLR)
    b_qa_t = np.ascontiguousarray(b_qa.reshape(-1, 128).T)
    b_kva_p = np.zeros(640, np.float32)
    b_kva_p[:NKV] = b_kva
    b_kva_t = np.ascontiguousarray(b_kva_p.reshape(5, 128).T)
    ln_qa_t = np.ascontiguousarray(w_qa_ln.reshape(-1, 128).T)
    ln_kva_t = np.ascontiguousarray(w_kva_ln.reshape(-1, 128).T)
    ones_col = np.ones((128, 1), np.float32)
    ones_c8 = np.ones((128, HPC * 2), np.float32)

    in_maps = []
    for c in range(N_CORES):
        b = c // TPG
        g = c % TPG
        hs = list(range(g * HPC, (g + 1) * HPC))
        # q_b columns: nope blocks by head then rope blocks by head
        qb_nope = np.concatenate([W_qb_h[h, :DN, :] for h in hs], 0)   # [512,QLR]
        qb_rope = np.concatenate([W_qb_h[h, DN:, :] for h in hs], 0)   # [256,QLR]
        w_qbT = np.ascontiguousarray(np.concatenate([qb_nope, qb_rope], 0).T)
        w_kvb_nT = np.ascontiguousarray(
            np.concatenate([W_kvb_h[h, :DN, :] for h in hs], 0).T)
        w_kvb_vT = np.ascontiguousarray(
            np.concatenate([W_kvb_h[h, DN:, :] for h in hs], 0).T)
        w_oT = np.ascontiguousarray(
            W_o[:, g * HPC * DV:(g + 1) * HPC * DV].T)
        in_maps.append({
            "xT": np.ascontiguousarray(hidden_states[b].T),
            "w_qaT": w_qaT, "w_kvaT": w_kvaT,
            "w_qbT": w_qbT, "w_kvb_nT": w_kvb_nT, "w_kvb_vT": w_kvb_vT,
            "w_oT": w_oT,
            "b_qa": b_qa_t, "b_kva": b_kva_t,
            "ln_qa": ln_qa_t, "ln_kva": ln_kva_t,
            "pos": np.ascontiguousarray(
                position_ids[b].astype(np.int32).reshape(1, S)),
            "inv_freq": inv_freq128,
            "p128": p128, "ones_col": ones_col, "ones_c8": ones_c8,
            "ident": ident, "masks": masks,
        })

    res = run_bass_kernel_spmd(nc, in_maps, list(range(N_CORES)))
    global LAST_RES
    LAST_RES = res

    out = np.zeros((B, S, HID), np.float32)
    for c in range(N_CORES):
        out[c // TPG] += res.results[c]["out"]
    return out


if __name__ == "__main__":
    import time
    t0 = time.time()
    build_kernel()
    print(f"build+compile: {time.time()-t0:.1f}s")


# revision 26
# speedup vs baseline: 1.0708x; 1.0708x over previous
"""DeepSeekV2-style MLA attention forward on 8 Trainium2 NeuronCores.

Sharding: 2-way data-parallel over batch x 4-way tensor-parallel over heads
(4 heads per core). The q_a projection is column-sharded across each batch's
TP group and AllGather'ed (with a tiny AllReduce for the rmsnorm sumsq);
kv_a is replicated. o_proj partial outputs are summed on the host.

Layout convention on device: activations live transposed as [feature, token]
so that every matmul is out^T[f_out, t] = lhsT(W^T tile).T @ rhs(x^T tile),
with weights pre-transposed on the host. All matmuls use the f32r
(reduced-precision fp32) PE path: 4x faster than fp32, ~1.4e-4 rel err.

Attention: scores are computed transposed s^T[k, q] (k on partitions), exp'd
on ScalarE without max-subtraction (scores are provably small here), masked
on the causal diagonal blocks, then AV uses p^T as the 512-wide moving
operand (out^T[dv, q]) with the softmax denominator from a ones-column
matmul; o_proj is fused into the same q-tile loop.
"""
import math
import sys

import numpy as np

try:
    import concourse.bass as bass  # noqa: F401
except ImportError:  # pragma: no cover
    sys.path.insert(0, "/opt/trn_rl_repo")

import concourse.bass as bass
import concourse.tile as tile
from concourse import bacc, mybir
from concourse.bass_utils import run_bass_kernel_spmd

# ---- problem dims (hardcoded per contest contract) ----
B, S, HID = 2, 2048, 2048
NH = 16
DN, DR, DV = 128, 64, 128
QD = DN + DR                       # 192
QLR, KVLR = 1536, 512
EPS = 1e-6
ROPE_BASE = 10000.0
SCALE = 1.0 / math.sqrt(QD)

N_CORES = 8
TPG = 4                            # TP group size (cores per batch)
HPC = NH // TPG                    # heads per core = 4

F32 = mybir.dt.float32
F32R = mybir.dt.float32r
I32 = mybir.dt.int32

NKV = KVLR + DR                    # 576 kv_a rows
T_TILE = 512                       # token tile (free dim)
NT = S // T_TILE                   # 4 token tiles
KB = S // 128                      # 16 key tiles of 128

QSH = QLR // TPG                   # 384 q_a rows per core
NFO_QSH = QSH // 128               # 3
NFO_KV = KVLR // 128               # 4
NFO_QA = QLR // 128                # 12
NHI = HID // 128                   # 16

TWO_PI = 2.0 * math.pi
MAGIC = np.float32(1.5 * 2**23)    # round-to-nearest-int magic constant

REPLICA_GROUPS = [[0, 1, 2, 3], [4, 5, 6, 7]]


def _cody_waite_consts():
    def trunc12(x):
        return np.frombuffer(
            (np.frombuffer(np.float32(x).tobytes(), np.uint32)
             & np.uint32(0xFFFFF000)).tobytes(), np.float32)[0]
    c1 = trunc12(np.float64(TWO_PI))
    c2 = trunc12(np.float64(TWO_PI) - np.float64(c1))
    c3 = np.float32(np.float64(TWO_PI) - np.float64(c1) - np.float64(c2))
    return float(c1), float(c2), float(c3)


CW1, CW2, CW3 = _cody_waite_consts()

_BUILD_CACHE = {}


def build_kernel(debug=False):
    key = bool(debug)
    if key in _BUILD_CACHE:
        return _BUILD_CACHE[key]

    nc = bacc.Bacc("TRN2", target_bir_lowering=False, debug=False,
                   num_devices=N_CORES)

    def din(name, shape, dt=F32R):
        return nc.dram_tensor(name, list(shape), dt, kind="ExternalInput").ap()

    # ---- per-core external inputs ----
    xT = din("xT", [HID, S])                       # hidden^T for this batch
    w_qaT = din("w_qaT", [HID, QSH])               # this core's q_a columns
    w_kvaT = din("w_kvaT", [HID, NKV])
    w_qbT = din("w_qbT", [QLR, HPC * QD])          # cols: nope h0..h3 | rope h0..h3
    w_kvb_nT = din("w_kvb_nT", [KVLR, HPC * DN])   # k_nope cols by head
    w_kvb_vT = din("w_kvb_vT", [KVLR, HPC * DV])   # v cols by head
    w_oT = din("w_oT", [HPC * DV, HID])
    b_qa = din("b_qa", [128, NFO_QSH], F32)        # this core's q_a bias slices
    b_kva = din("b_kva", [128, 5], F32)            # 576 padded to 640
    ln_qa = din("ln_qa", [128, NFO_QA], F32)       # full ln weights
    ln_kva = din("ln_kva", [128, NFO_KV], F32)
    pos = din("pos", [1, S], I32)
    inv_freq = din("inv_freq", [128, 1], F32)      # rope inv freqs, 4x repeated
    p128 = din("p128", [128, 128])                 # blockdiag(rotT, rotT) f32r
    ones_col = din("ones_col", [128, 1])           # f32r ones (sum matmuls)
    masks = din("masks", [4, 128, T_TILE])         # f32r causal diag masks

    out = nc.dram_tensor("out", [S, HID], F32, kind="ExternalOutput").ap()

    # ---- DRAM intermediates ----
    ikind = "ExternalOutput" if debug else "Internal"

    def dmid(name, shape, dt, shared=False):
        return nc.dram_tensor(
            name, list(shape), dt, kind=ikind,
            addr_space="Shared" if shared else "Local").ap()

    # collective in/out tensors must stay Internal (cannot be IO)
    qa_shard = nc.dram_tensor("qa_shard", [NT, QSH, T_TILE], F32).ap()
    qa_all = nc.dram_tensor("qa_all", [NT, QLR, T_TILE], F32).ap()
    ssq_part = nc.dram_tensor("ssq_part", [1, S], F32).ap()
    ssq_all = nc.dram_tensor("ssq_all", [1, S], F32).ap()
    kva_val = dmid("kva_val", [640, S], F32)       # kv_a pre-norm, 576 pad 640
    qT_nope = dmid("qT_nope", [HPC * DN, S], F32R)
    qT_rope = dmid("qT_rope", [HPC * DR, S], F32R)  # rope'd, by head
    k_nope = dmid("k_nope", [HPC * DN, S], F32R)
    k_rot = dmid("k_rot", [DR, S], F32R)           # shared rope'd key
    v_d = dmid("v_d", [KB, 128, HPC, DV], F32R)    # [tok, head, dv]
    if debug:
        attn_T = dmid("attn_T", [HPC * DV, S], F32R)
        dbg_sin = dmid("dbg_sin", [128, S], F32)
        dbg_cos = dmid("dbg_cos", [128, S], F32)
        dbg_rstd = dmid("dbg_rstd", [2, S], F32)

    with tile.TileContext(nc) as tc:
        with tc.tile_pool(name="const", bufs=1) as constp:
            bqa_t = constp.tile([128, NFO_QSH], F32)
            nc.sync.dma_start(bqa_t[:], b_qa[:])
            bkva_t = constp.tile([128, 5], F32)
            nc.sync.dma_start(bkva_t[:], b_kva[:])
            lnqa_t = constp.tile([128, NFO_QA], F32)
            nc.sync.dma_start(lnqa_t[:], ln_qa[:])
            lnkva_t = constp.tile([128, NFO_KV], F32)
            nc.sync.dma_start(lnkva_t[:], ln_kva[:])
            ones_t = constp.tile([128, 1], F32R)
            nc.sync.dma_start(ones_t[:], ones_col[:])
            p128_t = constp.tile([128, 128], F32R)
            nc.sync.dma_start(p128_t[:], p128[:])
            ivf_t = constp.tile([128, 1], F32)
            nc.sync.dma_start(ivf_t[:], inv_freq[:])
            # rstd broadcast tiles (qa & kv), filled in phase A
            rstd_qa_b = constp.tile([128, NT, T_TILE], F32)
            rstd_kv_b = constp.tile([128, NT, T_TILE], F32)
            # cos/sin tables, filled in phase R
            cos_t = constp.tile([128, NT, T_TILE], F32)
            sin_t = constp.tile([128, NT, T_TILE], F32)

            # ---------- phase R: rope cos/sin tables ----------
            with nc.named_scope("rope_tables"), \
                 tc.tile_pool(name="ropep", bufs=1) as rp:
                pos_i = rp.tile([1, S], I32)
                nc.sync.dma_start(pos_i[:], pos[:])
                pos_f = rp.tile([1, S], F32)
                nc.vector.tensor_copy(pos_f[:], pos_i[:])
                pos_b = rp.tile([128, S], F32)
                nc.gpsimd.partition_broadcast(pos_b[:], pos_f[:])
                freqs = rp.tile([128, S], F32)
                nc.vector.tensor_scalar_mul(freqs[:], pos_b[:], ivf_t[:])
                kr = rp.tile([128, S], F32)
                nc.vector.tensor_scalar(kr[:], freqs[:], 1.0 / TWO_PI,
                                        float(MAGIC), mybir.AluOpType.mult,
                                        mybir.AluOpType.add)
                nc.vector.tensor_scalar_sub(kr[:], kr[:], float(MAGIC))
                red = rp.tile([128, S], F32)
                nc.vector.cody_waite_cascade(red[:], freqs[:], kr[:],
                                             CW1, CW2, CW3)
                nc.scalar.activation(sin_t.rearrange("p n t -> p (n t)"),
                                     red[:], mybir.ActivationFunctionType.Sin)
                redc = rp.tile([128, S], F32)
                nc.vector.add_range_wrap(redc[:], red[:], math.pi / 2.0,
                                         math.pi, TWO_PI)
                nc.scalar.activation(cos_t.rearrange("p n t -> p (n t)"),
                                     redc[:], mybir.ActivationFunctionType.Sin)
                if debug:
                    nc.sync.dma_start(dbg_sin[:],
                                      sin_t.rearrange("p n t -> p (n t)"))
                    nc.sync.dma_start(dbg_cos[:],
                                      cos_t.rearrange("p n t -> p (n t)"))

            # ---------- phase A: q_a shard / kv_a + rmsnorm stats ----------
            xT_r = xT.rearrange("(hi p) s -> p hi s", p=128)
            kva_val_r = kva_val.rearrange("(f p) s -> p f s", p=128)
            w_qaT_r = w_qaT.rearrange("(hi p) f -> p hi f", p=128)
            w_kvaT_r = w_kvaT.rearrange("(hi p) f -> p hi f", p=128)

            with nc.named_scope("proj_a"), \
                 tc.tile_pool(name="xa", bufs=2) as xap, \
                 tc.tile_pool(name="wa", bufs=3) as wap, \
                 tc.tile_pool(name="va", bufs=3) as vap, \
                 tc.tile_pool(name="pa", bufs=3, space="PSUM") as pap, \
                 tc.tile_pool(name="ssp", bufs=2, space="PSUM") as ssp:
                for t in range(NT):
                    tsl = slice(t * T_TILE, (t + 1) * T_TILE)
                    xa = xap.tile([128, NHI, T_TILE], F32R, tag="xa")
                    nc.sync.dma_start(xa[:], xT_r[:, :, tsl])
                    ss_qa = ssp.tile([1, T_TILE], F32, tag="ssqa")
                    ss_kv = ssp.tile([1, T_TILE], F32, tag="sskv")
                    for proj in range(2):
                        nfo = NFO_QSH if proj == 0 else 5
                        wsrc = w_qaT_r if proj == 0 else w_kvaT_r
                        bias_t = bqa_t if proj == 0 else bkva_t
                        for fo in range(nfo):
                            m = 128 if not (proj == 1 and fo == 4) else 64
                            wt = wap.tile([128, NHI, 128], F32R, tag="wt")
                            nc.sync.dma_start(
                                wt[:, :, :m], wsrc[:, :, fo * 128:fo * 128 + m])
                            ps = pap.tile([m, T_TILE], F32, tag="acc")
                            for hi in range(NHI):
                                nc.tensor.matmul(
                                    ps[:], wt[:, hi, :m], xa[:, hi, :],
                                    start=(hi == 0), stop=(hi == NHI - 1))
                            val = vap.tile([128, T_TILE], F32, tag="val")
                            nc.vector.tensor_scalar_add(
                                val[:m], ps[:], bias_t[:m, fo:fo + 1])
                            if proj == 0:
                                nc.sync.dma_start(
                                    qa_shard[t, fo * 128:(fo + 1) * 128, :],
                                    val[:])
                            else:
                                nc.sync.dma_start(
                                    kva_val_r[:m, fo, tsl], val[:m])
                            if not (proj == 1 and fo == 4):
                                sq = vap.tile([128, T_TILE], F32R, tag="sq")
                                nc.vector.tensor_tensor(
                                    sq[:], val[:], val[:],
                                    mybir.AluOpType.mult)
                                sst = ss_qa if proj == 0 else ss_kv
                                nc.tensor.matmul(
                                    sst[:], ones_t[:], sq[:],
                                    start=(fo == 0),
                                    stop=(fo == nfo - 1 - (proj == 1)))
                    # local kv rstd; qa sumsq is partial -> AllReduce later
                    ssq_sb = vap.tile([1, T_TILE], F32, tag="ssq_sb")
                    nc.vector.tensor_copy(ssq_sb[:], ss_qa[:])
                    nc.sync.dma_start(ssq_part[:, tsl], ssq_sb[:])
                    ms = vap.tile([1, T_TILE], F32, tag="ms")
                    nc.vector.tensor_scalar(
                        ms[:], ss_kv[:], 1.0 / KVLR, EPS,
                        mybir.AluOpType.mult, mybir.AluOpType.add)
                    std = vap.tile([1, T_TILE], F32, tag="std")
                    nc.scalar.activation(std[:], ms[:],
                                         mybir.ActivationFunctionType.Sqrt)
                    rstd = vap.tile([1, T_TILE], F32, tag="rstd")
                    nc.vector.reciprocal(rstd[:], std[:])
                    nc.gpsimd.partition_broadcast(rstd_kv_b[:, t, :], rstd[:])
                    if debug:
                        nc.sync.dma_start(dbg_rstd[1:2, tsl], rstd[:])
                    # AllGather this token-tile's q_a shard
                    nc.gpsimd.collective_compute(
                        "AllGather", mybir.AluOpType.bypass,
                        replica_groups=REPLICA_GROUPS,
                        ins=[qa_shard[t]], outs=[qa_all[t]])

                # AllReduce the q_a sumsq partials, then qa rstd per tile
                nc.gpsimd.collective_compute(
                    "AllReduce", mybir.AluOpType.add,
                    replica_groups=REPLICA_GROUPS,
                    ins=[ssq_part[:]], outs=[ssq_all[:]])
                for t in range(NT):
                    tsl = slice(t * T_TILE, (t + 1) * T_TILE)
                    ssl = vap.tile([1, T_TILE], F32, tag="ssl")
                    nc.sync.dma_start(ssl[:], ssq_all[:, tsl])
                    ms = vap.tile([1, T_TILE], F32, tag="ms")
                    nc.vector.tensor_scalar(
                        ms[:], ssl[:], 1.0 / QLR, EPS,
                        mybir.AluOpType.mult, mybir.AluOpType.add)
                    std = vap.tile([1, T_TILE], F32, tag="std")
                    nc.scalar.activation(std[:], ms[:],
                                         mybir.ActivationFunctionType.Sqrt)
                    rstd = vap.tile([1, T_TILE], F32, tag="rstd")
                    nc.vector.reciprocal(rstd[:], std[:])
                    nc.gpsimd.partition_broadcast(rstd_qa_b[:, t, :], rstd[:])
                    if debug:
                        nc.sync.dma_start(dbg_rstd[0:1, tsl], rstd[:])

            # ---------- phase B: q_b / kv_b / rope ----------
            qT_nope_r = qT_nope.rearrange("(f p) s -> p f s", p=128)
            qT_rope_r2 = qT_rope.rearrange("(f p) s -> p f s", p=128)
            k_nope_r = k_nope.rearrange("(f p) s -> p f s", p=128)
            w_qbT_r = w_qbT.rearrange("(fi p) f -> p fi f", p=128)
            w_kvb_nT_r = w_kvb_nT.rearrange("(fi p) f -> p fi f", p=128)
            w_kvb_vT_r = w_kvb_vT.rearrange("(fi p) f -> p fi f", p=128)

            NQB = HPC * QD // 128  # 6 output tiles (4 nope + 2 rope-pair)

            with nc.named_scope("proj_b"), \
                 tc.tile_pool(name="wb", bufs=3) as wbp, \
                 tc.tile_pool(name="rhb", bufs=2) as rhbp, \
                 tc.tile_pool(name="evb", bufs=2) as evbp, \
                 tc.tile_pool(name="pb", bufs=2, space="PSUM") as pbp:
                wv_t = wbp.tile([128, NFO_KV, HPC * DV], F32R, name="wv_t")
                nc.sync.dma_start(wv_t[:], w_kvb_vT_r[:])
                for t in range(NT):
                    tsl = slice(t * T_TILE, (t + 1) * T_TILE)
                    # load + normalize q_a rhs (from the AllGather'ed q_a)
                    qa_rhs = rhbp.tile([128, NFO_QA, T_TILE], F32R, tag="qarhs")
                    for f in range(NFO_QA):
                        valt = evbp.tile([128, T_TILE], F32, tag="ld")
                        nc.sync.dma_start(
                            valt[:], qa_all[t, f * 128:(f + 1) * 128, :])
                        nc.vector.scalar_tensor_tensor(
                            qa_rhs[:, f, :], valt[:], lnqa_t[:, f:f + 1],
                            rstd_qa_b[:, t, :],
                            mybir.AluOpType.mult, mybir.AluOpType.mult)
                    ckv_rhs = rhbp.tile([128, NFO_KV, T_TILE], F32R, tag="ckvrhs")
                    for f in range(NFO_KV):
                        valt = evbp.tile([128, T_TILE], F32, tag="ld")
                        nc.sync.dma_start(valt[:], kva_val_r[:, f, tsl])
                        nc.vector.scalar_tensor_tensor(
                            ckv_rhs[:, f, :], valt[:], lnkva_t[:, f:f + 1],
                            rstd_kv_b[:, t, :],
                            mybir.AluOpType.mult, mybir.AluOpType.mult)

                    # q_b: 4 nope tiles then 2 rope-pair tiles
                    for fo in range(NQB):
                        wt = wbp.tile([128, NFO_QA, 128], F32R, tag="wqb")
                        nc.sync.dma_start(
                            wt[:], w_qbT_r[:, :, fo * 128:(fo + 1) * 128])
                        ps = pbp.tile([128, T_TILE], F32, tag="qb")
                        for fi in range(NFO_QA):
                            nc.tensor.matmul(ps[:], wt[:, fi, :],
                                             qa_rhs[:, fi, :],
                                             start=(fi == 0),
                                             stop=(fi == NFO_QA - 1))
                        if fo < HPC:  # nope
                            ev = evbp.tile([128, T_TILE], F32R, tag="evr")
                            nc.scalar.activation(
                                ev[:], ps[:],
                                mybir.ActivationFunctionType.Copy)
                            nc.sync.dma_start(qT_nope_r[:, fo, tsl], ev[:])
                        else:  # rope pair: rows = heads (2j, 2j+1)
                            qpe = evbp.tile([128, T_TILE], F32R, tag="evr")
                            nc.scalar.activation(
                                qpe[:], ps[:],
                                mybir.ActivationFunctionType.Copy)
                            rps = pbp.tile([128, T_TILE], F32, tag="rot",
                                           bufs=1)
                            nc.tensor.matmul(rps[:], p128_t[:], qpe[:],
                                             start=True, stop=True)
                            tmp = evbp.tile([128, T_TILE], F32, tag="tmp")
                            nc.vector.tensor_tensor(tmp[:], cos_t[:, t, :],
                                                    qpe[:],
                                                    mybir.AluOpType.mult)
                            rot = evbp.tile([128, T_TILE], F32, tag="rot2")
                            nc.vector.tensor_tensor(rot[:], sin_t[:, t, :],
                                                    rps[:],
                                                    mybir.AluOpType.mult)
                            qro = evbp.tile([128, T_TILE], F32R, tag="evr2")
                            nc.vector.tensor_tensor(qro[:], tmp[:], rot[:],
                                                    mybir.AluOpType.add)
                            j = fo - HPC
                            nc.sync.dma_start(qT_rope_r2[:, j, tsl], qro[:])

                    # kv_b nope
                    for fo in range(HPC):
                        wt = wbp.tile([128, NFO_KV, 128], F32R, tag="wkn")
                        nc.sync.dma_start(
                            wt[:], w_kvb_nT_r[:, :, fo * 128:(fo + 1) * 128])
                        ps = pbp.tile([128, T_TILE], F32, tag="qb")
                        for fi in range(NFO_KV):
                            nc.tensor.matmul(ps[:], wt[:, fi, :],
                                             ckv_rhs[:, fi, :],
                                             start=(fi == 0),
                                             stop=(fi == NFO_KV - 1))
                        ev = evbp.tile([128, T_TILE], F32R, tag="evr")
                        nc.scalar.activation(ev[:], ps[:],
                                             mybir.ActivationFunctionType.Copy)
                        nc.sync.dma_start(k_nope_r[:, fo, tsl], ev[:])

                    # v (un-transposed): [tok, head*dv]
                    for ts in range(T_TILE // 128):
                        kb = t * 4 + ts
                        ps = pbp.tile([128, HPC * DV], F32, tag="vps", bufs=2)
                        for fi in range(NFO_KV):
                            nc.tensor.matmul(
                                ps[:],
                                ckv_rhs[:, fi, ts * 128:(ts + 1) * 128],
                                wv_t[:, fi, :],
                                start=(fi == 0), stop=(fi == NFO_KV - 1))
                        vt = evbp.tile([128, HPC, DV], F32R, tag="vt")
                        nc.scalar.activation(
                            vt[:],
                            ps[:].rearrange("p (h d) -> p h d", h=HPC),
                            mybir.ActivationFunctionType.Copy)
                        nc.sync.dma_start(v_d[kb], vt[:])

                    # k_pe rope (shared across heads)
                    kpe = evbp.tile([64, T_TILE], F32R, tag="kpe")
                    valt = evbp.tile([64, T_TILE], F32, tag="ld64")
                    nc.sync.dma_start(valt[:],
                                      kva_val[KVLR:KVLR + DR, tsl])
                    nc.vector.tensor_copy(kpe[:], valt[:])
                    rps = pbp.tile([64, T_TILE], F32, tag="rotk", bufs=1)
                    nc.tensor.matmul(rps[:], p128_t[0:64, 0:64], kpe[:],
                                     start=True, stop=True)
                    tmp = evbp.tile([64, T_TILE], F32, tag="tmpk")
                    nc.vector.tensor_tensor(tmp[:], cos_t[0:64, t, :], kpe[:],
                                            mybir.AluOpType.mult)
                    rot = evbp.tile([64, T_TILE], F32, tag="rotk2")
                    nc.vector.tensor_tensor(rot[:], sin_t[0:64, t, :], rps[:],
                                            mybir.AluOpType.mult)
                    kro = evbp.tile([64, T_TILE], F32R, tag="kro")
                    nc.vector.tensor_tensor(kro[:], tmp[:], rot[:],
                                            mybir.AluOpType.add)
                    nc.sync.dma_start(k_rot[:, tsl], kro[:])

            # ---------- phase C: attention + fused o_proj ----------
            w_oT_r = w_oT.rearrange("(fs p) hid -> p fs hid", p=128)
            qT_rope_r = qT_rope.rearrange("(f p) s -> p f s", p=64)
            with nc.named_scope("attn"), \
                 tc.tile_pool(name="kv", bufs=1) as kvp, \
                 tc.tile_pool(name="qrh", bufs=3) as qrhp, \
                 tc.tile_pool(name="pt", bufs=4) as ptp, \
                 tc.tile_pool(name="ao", bufs=2) as aop, \
                 tc.tile_pool(name="oe", bufs=3) as oep, \
                 tc.tile_pool(name="sps", bufs=2, space="PSUM") as spsp, \
                 tc.tile_pool(name="avs", bufs=2, space="PSUM") as avsp, \
                 tc.tile_pool(name="lps", bufs=2, space="PSUM") as lpsp, \
                 tc.tile_pool(name="pos_", bufs=2, space="PSUM") as posp:
                masks_t = kvp.tile([128, 4, T_TILE], F32R, name="masks_t")
                nc.sync.dma_start(masks_t[:], masks.rearrange("j p t -> p j t"))
                krot_sb = kvp.tile([64, S], F32R, name="krot_sb")
                nc.sync.dma_start(krot_sb[:], k_rot[:])
                wo_sb = kvp.tile([128, HPC, HID], F32R, name="wo_sb")
                nc.sync.dma_start(wo_sb[:], w_oT_r[:])
                kn_sb = kvp.tile([128, HPC, S], F32R, name="kn_sb")
                vh_sb = kvp.tile([128, HPC, KB, DV], F32R, name="vh_sb")
                for h in range(HPC):
                    nc.sync.dma_start(kn_sb[:, h, :], k_nope_r[:, h, :])
                    nc.sync.dma_start(
                        vh_sb[:, h, :, :],
                        v_d[:, :, h:h + 1, :].rearrange(
                            "kb p one d -> p kb (one d)"))
                for qt in range(NT):
                    qsl = slice(qt * T_TILE, (qt + 1) * T_TILE)
                    at_full = aop.tile([128, HPC, T_TILE], F32R, tag="atf")
                    nkb = 4 * qt + 4
                    for h in range(HPC):
                        qn_rhs = qrhp.tile([128, T_TILE], F32R, tag="qn")
                        nc.sync.dma_start(qn_rhs[:], qT_nope_r[:, h, qsl])
                        qr_rhs = qrhp.tile([64, T_TILE], F32R, tag="qr")
                        nc.sync.dma_start(qr_rhs[:], qT_rope_r[:, h, qsl])
                        av_ps = avsp.tile([128, T_TILE], F32, tag="av")
                        l_ps = lpsp.tile([1, T_TILE], F32, tag="l")
                        for kb in range(nkb):
                            sps = spsp.tile([128, T_TILE], F32, tag="s")
                            nc.tensor.matmul(
                                sps[:],
                                kn_sb[:, h, kb * 128:(kb + 1) * 128],
                                qn_rhs[:], start=True, stop=False)
                            nc.tensor.matmul(
                                sps[:], krot_sb[:, kb * 128:(kb + 1) * 128],
                                qr_rhs[:], start=False, stop=True)
                            pt = ptp.tile([128, T_TILE], F32R, tag="p")
                            nc.scalar.activation(
                                pt[:], sps[:],
                                mybir.ActivationFunctionType.Exp, scale=SCALE)
                            j = kb - 4 * qt
                            if j >= 0:
                                nc.vector.tensor_tensor(
                                    pt[:], pt[:], masks_t[:, j, :],
                                    mybir.AluOpType.mult)
                            nc.tensor.matmul(
                                av_ps[:], vh_sb[:, h, kb, :], pt[:],
                                start=(kb == 0), stop=(kb == nkb - 1))
                            nc.tensor.matmul(
                                l_ps[:], ones_t[:], pt[:],
                                start=(kb == 0), stop=(kb == nkb - 1))
                        rec = qrhp.tile([1, T_TILE], F32, tag="rec")
                        nc.vector.reciprocal(rec[:], l_ps[:])
                        rec_b = qrhp.tile([128, T_TILE], F32, tag="recb")
                        nc.gpsimd.partition_broadcast(rec_b[:], rec[:])
                        nc.vector.tensor_tensor(
                            at_full[:, h, :], av_ps[:], rec_b[:],
                            mybir.AluOpType.mult)
                        if debug:
                            nc.sync.dma_start(attn_T.rearrange(
                                "(f p) s -> p f s", p=128)[:, h, qsl],
                                at_full[:, h, :])
                    # fused o_proj for this q-tile
                    for ts in range(T_TILE // 128):
                        tok0 = qt * T_TILE + ts * 128
                        for ho in range(HID // T_TILE):
                            ps = posp.tile([128, T_TILE], F32, tag="po")
                            for fs in range(HPC):
                                nc.tensor.matmul(
                                    ps[:],
                                    at_full[:, fs, ts * 128:(ts + 1) * 128],
                                    wo_sb[:, fs,
                                          ho * T_TILE:(ho + 1) * T_TILE],
                                    start=(fs == 0), stop=(fs == HPC - 1))
                            oe = oep.tile([128, T_TILE], F32, tag="oe")
                            nc.scalar.activation(
                                oe[:], ps[:],
                                mybir.ActivationFunctionType.Copy)
                            nc.sync.dma_start(
                                out[tok0:tok0 + 128,
                                    ho * T_TILE:(ho + 1) * T_TILE],
                                oe[:])

    nc.compile()
    _BUILD_CACHE[key] = nc
    return nc


def _host_consts():
    ivf = (1.0 / (ROPE_BASE ** (np.arange(0, DR, 2, dtype=np.float64) / DR)))
    ivf = ivf.astype(np.float32)                       # [32]
    inv_freq128 = np.tile(ivf, 4).reshape(128, 1)

    rot = np.zeros((DR, DR), np.float32)               # rot(x) = P @ x
    for d in range(32):
        rot[d, d + 32] = -1.0
        rot[d + 32, d] = 1.0
    rotT = rot.T
    p128 = np.zeros((128, 128), np.float32)
    p128[:64, :64] = rotT
    p128[64:, 64:] = rotT

    kk = np.arange(128)[None, :, None]                 # [1,128,1]
    jj = np.arange(4)[:, None, None]                   # [4,1,1]
    qq = np.arange(T_TILE)[None, None, :]              # [1,1,512]
    masks = ((jj * 128 + kk) <= qq).astype(np.float32)  # [4,128,512]

    return inv_freq128, p128, masks


LAST_RES = None


def kernel(_debug=False, **inputs):
    hidden_states = np.asarray(inputs["hidden_states"], np.float32)
    position_ids = np.asarray(inputs["position_ids"])
    W_qa = np.asarray(inputs["W_qa"], np.float32)
    b_qa = np.asarray(inputs["b_qa"], np.float32)
    w_qa_ln = np.asarray(inputs["w_qa_ln"], np.float32)
    W_qb = np.asarray(inputs["W_qb"], np.float32)
    W_kva = np.asarray(inputs["W_kva"], np.float32)
    b_kva = np.asarray(inputs["b_kva"], np.float32)
    w_kva_ln = np.asarray(inputs["w_kva_ln"], np.float32)
    W_kvb = np.asarray(inputs["W_kvb"], np.float32)
    W_o = np.asarray(inputs["W_o"], np.float32)

    nc = build_kernel(debug=_debug)

    inv_freq128, p128, masks = _host_consts()

    w_kvaT = np.ascontiguousarray(W_kva.T)
    W_qb_h = W_qb.reshape(NH, QD, QLR)
    W_kvb_h = W_kvb.reshape(NH, DN + DV, KVLR)
    b_kva_p = np.zeros(640, np.float32)
    b_kva_p[:NKV] = b_kva
    b_kva_t = np.ascontiguousarray(b_kva_p.reshape(5, 128).T)
    ln_qa_t = np.ascontiguousarray(w_qa_ln.reshape(-1, 128).T)
    ln_kva_t = np.ascontiguousarray(w_kva_ln.reshape(-1, 128).T)
    ones_col = np.ones((128, 1), np.float32)

    in_maps = []
    for c in range(N_CORES):
        b = c // TPG
        g = c % TPG
        hs = list(range(g * HPC, (g + 1) * HPC))
        w_qaT = np.ascontiguousarray(W_qa[g * QSH:(g + 1) * QSH, :].T)
        b_qa_t = np.ascontiguousarray(
            b_qa[g * QSH:(g + 1) * QSH].reshape(NFO_QSH, 128).T)
        # q_b columns: nope blocks by head then rope blocks by head
        qb_nope = np.concatenate([W_qb_h[h, :DN, :] for h in hs], 0)
        qb_rope = np.concatenate([W_qb_h[h, DN:, :] for h in hs], 0)
        w_qbT = np.ascontiguousarray(np.concatenate([qb_nope, qb_rope], 0).T)
        w_kvb_nT = np.ascontiguousarray(
            np.concatenate([W_kvb_h[h, :DN, :] for h in hs], 0).T)
        w_kvb_vT = np.ascontiguousarray(
            np.concatenate([W_kvb_h[h, DN:, :] for h in hs], 0).T)
        w_oT = np.ascontiguousarray(
            W_o[:, g * HPC * DV:(g + 1) * HPC * DV].T)
        in_maps.append({
            "xT": np.ascontiguousarray(hidden_states[b].T),
            "w_qaT": w_qaT, "w_kvaT": w_kvaT,
            "w_qbT": w_qbT, "w_kvb_nT": w_kvb_nT, "w_kvb_vT": w_kvb_vT,
            "w_oT": w_oT,
            "b_qa": b_qa_t, "b_kva": b_kva_t,
            "ln_qa": ln_qa_t, "ln_kva": ln_kva_t,
            "pos": np.ascontiguousarray(
                position_ids[b].astype(np.int32).reshape(1, S)),
            "inv_freq": inv_freq128,
            "p128": p128, "ones_col": ones_col,
            "masks": masks,
        })

    res = run_bass_kernel_spmd(nc, in_maps, list(range(N_CORES)))
    global LAST_RES
    LAST_RES = res

    out = np.zeros((B, S, HID), np.float32)
    for c in range(N_CORES):
        out[c // TPG] += res.results[c]["out"]
    return out


if __name__ == "__main__":
    import time
    t0 = time.time()
    build_kernel()
    print(f"build+compile: {time.time()-t0:.1f}s")


# revision 36
# speedup vs baseline: 1.1521x; 1.0759x over previous
"""DeepSeekV2-style MLA attention forward on 8 Trainium2 NeuronCores.

Sharding: 2-way data-parallel over batch x 4-way tensor-parallel over heads
(4 heads per core). The shared low-rank q_a/kv_a projections are sharded
over TOKENS within each batch's TP group: each core projects+rmsnorms its
own quarter of the sequence fully locally, then one AllGather per tensor
(normalized q_a, and c_kv_norm|k_rot combined) replicates them. o_proj
partial outputs are summed on the host (TP unshard).

Layout convention on device: activations live transposed as [feature, token]
so that every matmul is out^T[f_out, t] = lhsT(W^T tile).T @ rhs(x^T tile),
with weights pre-transposed on the host. All matmuls use the f32r
(reduced-precision fp32) PE path: 4x faster than fp32, ~1.4e-4 rel err.

Attention: scores are computed transposed s^T[k, q] (k on partitions), exp'd
on ScalarE without max-subtraction (scores are provably small here), masked
on the causal diagonal blocks, then AV uses p^T as the 512-wide moving
operand (out^T[dv, q]) with the softmax denominator from a ones-column
matmul; o_proj is fused into the same q-tile loop.
"""
import math
import sys

import numpy as np

try:
    import concourse.bass as bass  # noqa: F401
except ImportError:  # pragma: no cover
    sys.path.insert(0, "/opt/trn_rl_repo")

import concourse.bass as bass
import concourse.tile as tile
from concourse import bacc, mybir
from concourse.bass_utils import run_bass_kernel_spmd

# ---- problem dims (hardcoded per contest contract) ----
B, S, HID = 2, 2048, 2048
NH = 16
DN, DR, DV = 128, 64, 128
QD = DN + DR                       # 192
QLR, KVLR = 1536, 512
EPS = 1e-6
ROPE_BASE = 10000.0
SCALE = 1.0 / math.sqrt(QD)

N_CORES = 8
TPG = 4                            # TP group size (cores per batch)
HPC = NH // TPG                    # heads per core = 4

F32 = mybir.dt.float32
F32R = mybir.dt.float32r
I32 = mybir.dt.int32

NKV = KVLR + DR                    # 576 kv_a rows
T_TILE = 512                       # token tile (free dim)
NT = S // T_TILE                   # 4 token tiles
KB = S // 128                      # 16 key tiles of 128

NFO_KV = KVLR // 128               # 4
NFO_QA = QLR // 128                # 12
NHI = HID // 128                   # 16

TWO_PI = 2.0 * math.pi
MAGIC = np.float32(1.5 * 2**23)    # round-to-nearest-int magic constant

REPLICA_GROUPS = [[0, 1, 2, 3], [4, 5, 6, 7]]


def _cody_waite_consts():
    def trunc12(x):
        return np.frombuffer(
            (np.frombuffer(np.float32(x).tobytes(), np.uint32)
             & np.uint32(0xFFFFF000)).tobytes(), np.float32)[0]
    c1 = trunc12(np.float64(TWO_PI))
    c2 = trunc12(np.float64(TWO_PI) - np.float64(c1))
    c3 = np.float32(np.float64(TWO_PI) - np.float64(c1) - np.float64(c2))
    return float(c1), float(c2), float(c3)


CW1, CW2, CW3 = _cody_waite_consts()

_BUILD_CACHE = {}


def build_kernel(debug=False):
    key = bool(debug)
    if key in _BUILD_CACHE:
        return _BUILD_CACHE[key]

    nc = bacc.Bacc("TRN2", target_bir_lowering=False, debug=False,
                   num_devices=N_CORES)

    def din(name, shape, dt=F32R):
        return nc.dram_tensor(name, list(shape), dt, kind="ExternalInput").ap()

    # ---- per-core external inputs ----
    xTl = din("xTl", [HID, T_TILE])                # hidden^T, LOCAL tokens
    w_qaT = din("w_qaT", [HID, QLR])
    w_kvaT = din("w_kvaT", [HID, NKV])
    w_qbT = din("w_qbT", [QLR, HPC * QD])          # cols: nope h0..h3 | rope h0..h3
    w_kvb_nT = din("w_kvb_nT", [KVLR, HPC * DN])   # k_nope cols by head
    w_kvb_vT = din("w_kvb_vT", [KVLR, HPC * DV])   # v cols by head
    w_oT = din("w_oT", [HPC * DV, HID])
    b_qa = din("b_qa", [128, NFO_QA], F32)
    b_kva = din("b_kva", [128, 5], F32)            # 576 padded to 640
    ln_qa = din("ln_qa", [128, NFO_QA], F32)
    ln_kva = din("ln_kva", [128, NFO_KV], F32)
    pos = din("pos", [1, S], I32)                  # full positions (for q rope)
    pos_l = din("pos_l", [1, T_TILE], I32)         # local positions (k_pe rope)
    inv_freq = din("inv_freq", [128, 1], F32)      # rope inv freqs, 4x repeated
    p128 = din("p128", [128, 128])                 # blockdiag(rotT, rotT) f32r
    ones_col = din("ones_col", [128, 1])           # f32r ones (sum matmuls)
    masks = din("masks", [4, 128, T_TILE])         # f32r causal diag masks

    out = nc.dram_tensor("out", [S, HID], F32, kind="ExternalOutput").ap()

    # ---- DRAM intermediates ----
    ikind = "ExternalOutput" if debug else "Internal"

    def dmid(name, shape, dt):
        return nc.dram_tensor(name, list(shape), dt, kind=ikind).ap()

    # collective in/out tensors must stay Internal (cannot be IO)
    qa_sh = nc.dram_tensor("qa_sh", [QLR, T_TILE], F32R).ap()
    qa_all = nc.dram_tensor("qa_all", [NT, QLR, T_TILE], F32R).ap()
    ckv_sh = nc.dram_tensor("ckv_sh", [NKV, T_TILE], F32R).ap()  # norm| k_rot
    ckv_all = nc.dram_tensor("ckv_all", [NT, NKV, T_TILE], F32R).ap()

    qT_nope = dmid("qT_nope", [HPC * DN, S], F32R)
    qT_rope = dmid("qT_rope", [HPC * DR, S], F32R)  # rope'd, by head
    if debug:
        dbg_kn = dmid("dbg_kn", [HPC * DN, S], F32R)
        dbg_v = dmid("dbg_v", [KB, 128, HPC, DV], F32R)
        dbg_krot = dmid("dbg_krot", [DR, S], F32R)
        attn_T = dmid("attn_T", [HPC * DV, S], F32R)
        dbg_sin = dmid("dbg_sin", [128, S], F32)
        dbg_cos = dmid("dbg_cos", [128, S], F32)

    with tile.TileContext(nc) as tc:
        with tc.tile_pool(name="const", bufs=1) as constp:
            bqa_t = constp.tile([128, NFO_QA], F32)
            nc.sync.dma_start(bqa_t[:], b_qa[:])
            bkva_t = constp.tile([128, 5], F32)
            nc.sync.dma_start(bkva_t[:], b_kva[:])
            lnqa_t = constp.tile([128, NFO_QA], F32)
            nc.sync.dma_start(lnqa_t[:], ln_qa[:])
            lnkva_t = constp.tile([128, NFO_KV], F32)
            nc.sync.dma_start(lnkva_t[:], ln_kva[:])
            ones_t = constp.tile([128, 1], F32R)
            nc.sync.dma_start(ones_t[:], ones_col[:])
            p128_t = constp.tile([128, 128], F32R)
            nc.sync.dma_start(p128_t[:], p128[:])
            ivf_t = constp.tile([128, 1], F32)
            nc.sync.dma_start(ivf_t[:], inv_freq[:])

            def rope_tables(pos_ap, n, cos_dst, sin_dst, rp, tag):
                """Build cos/sin [128, n] tables from int32 positions [1, n]."""
                pos_i = rp.tile([1, n], I32, name=f"pos_i_{tag}")
                nc.sync.dma_start(pos_i[:], pos_ap[:])
                pos_f = rp.tile([1, n], F32, name=f"pos_f_{tag}")
                nc.vector.tensor_copy(pos_f[:], pos_i[:])
                pos_b = rp.tile([128, n], F32, name=f"pos_b_{tag}")
                nc.gpsimd.partition_broadcast(pos_b[:], pos_f[:])
                freqs = rp.tile([128, n], F32, name=f"freqs_{tag}")
                nc.vector.tensor_scalar_mul(freqs[:], pos_b[:], ivf_t[:])
                kr = rp.tile([128, n], F32, name=f"kr_{tag}")
                nc.vector.tensor_scalar(kr[:], freqs[:], 1.0 / TWO_PI,
                                        float(MAGIC), mybir.AluOpType.mult,
                                        mybir.AluOpType.add)
                nc.vector.tensor_scalar_sub(kr[:], kr[:], float(MAGIC))
                red = rp.tile([128, n], F32, name=f"red_{tag}")
                nc.vector.cody_waite_cascade(red[:], freqs[:], kr[:],
                                             CW1, CW2, CW3)
                nc.scalar.activation(sin_dst, red[:],
                                     mybir.ActivationFunctionType.Sin)
                redc = rp.tile([128, n], F32, name=f"redc_{tag}")
                nc.vector.add_range_wrap(redc[:], red[:], math.pi / 2.0,
                                         math.pi, TWO_PI)
                nc.scalar.activation(cos_dst, redc[:],
                                     mybir.ActivationFunctionType.Sin)

            # ---------- phase A: local-token q_a / kv_a + rmsnorm + k rope --
            xTl_r = xTl.rearrange("(hi p) s -> p hi s", p=128)
            w_qaT_r = w_qaT.rearrange("(hi p) f -> p hi f", p=128)
            w_kvaT_r = w_kvaT.rearrange("(hi p) f -> p hi f", p=128)
            qa_sh_r = qa_sh.rearrange("(f p) s -> p f s", p=128)

            with nc.named_scope("proj_a"), \
                 tc.tile_pool(name="ap_", bufs=1) as ap_, \
                 tc.tile_pool(name="wa", bufs=2) as wap, \
                 tc.tile_pool(name="va", bufs=1) as vap, \
                 tc.tile_pool(name="pa", bufs=3, space="PSUM") as pap, \
                 tc.tile_pool(name="ssp", bufs=1, space="PSUM") as ssp:
                # local rope tables for k_pe
                cos_l = ap_.tile([128, T_TILE], F32)
                sin_l = ap_.tile([128, T_TILE], F32)
                rope_tables(pos_l, T_TILE, cos_l[:], sin_l[:], ap_, "loc")

                xa = ap_.tile([128, NHI, T_TILE], F32R)
                nc.sync.dma_start(xa[:], xTl_r[:])
                val_qa = ap_.tile([128, NFO_QA, T_TILE], F32)
                val_kv = ap_.tile([128, 5, T_TILE], F32)
                ss_qa = ssp.tile([1, T_TILE], F32, name="ss_qa")
                ss_kv = ssp.tile([1, T_TILE], F32, name="ss_kv")
                for proj in range(2):
                    nfo = NFO_QA if proj == 0 else 5
                    wsrc = w_qaT_r if proj == 0 else w_kvaT_r
                    bias_t = bqa_t if proj == 0 else bkva_t
                    vdst = val_qa if proj == 0 else val_kv
                    for fo in range(nfo):
                        m = 128 if not (proj == 1 and fo == 4) else 64
                        wt = wap.tile([128, NHI, 128], F32R, tag="wt")
                        nc.sync.dma_start(
                            wt[:, :, :m], wsrc[:, :, fo * 128:fo * 128 + m])
                        ps = pap.tile([m, T_TILE], F32, tag="acc")
                        for hi in range(NHI):
                            nc.tensor.matmul(
                                ps[:], wt[:, hi, :m], xa[:, hi, :],
                                start=(hi == 0), stop=(hi == NHI - 1))
                        nc.vector.tensor_scalar_add(
                            vdst[:m, fo, :], ps[:], bias_t[:m, fo:fo + 1])
                        if not (proj == 1 and fo == 4):
                            sq = vap.tile([128, T_TILE], F32R, tag="sq")
                            nc.vector.tensor_tensor(
                                sq[:], vdst[:, fo, :], vdst[:, fo, :],
                                mybir.AluOpType.mult)
                            sst = ss_qa if proj == 0 else ss_kv
                            nc.tensor.matmul(
                                sst[:], ones_t[:], sq[:],
                                start=(fo == 0),
                                stop=(fo == nfo - 1 - (proj == 1)))
                # rstd (fully local), normalize, then AllGather
                for proj in range(2):
                    d = QLR if proj == 0 else KVLR
                    nfo = NFO_QA if proj == 0 else NFO_KV
                    sst = ss_qa if proj == 0 else ss_kv
                    ln_t = lnqa_t if proj == 0 else lnkva_t
                    vsrc = val_qa if proj == 0 else val_kv
                    ms = vap.tile([1, T_TILE], F32, tag="ms")
                    nc.vector.tensor_scalar(
                        ms[:], sst[:], 1.0 / d, EPS,
                        mybir.AluOpType.mult, mybir.AluOpType.add)
                    std = vap.tile([1, T_TILE], F32, tag="std")
                    nc.scalar.activation(std[:], ms[:],
                                         mybir.ActivationFunctionType.Sqrt)
                    rstd = vap.tile([1, T_TILE], F32, tag="rstd")
                    nc.vector.reciprocal(rstd[:], std[:])
                    rstd_b = vap.tile([128, T_TILE], F32, tag="rstdb")
                    nc.gpsimd.partition_broadcast(rstd_b[:], rstd[:])
                    for f in range(nfo):
                        nrm = vap.tile([128, T_TILE], F32R, tag="nrm")
                        nc.vector.scalar_tensor_tensor(
                            nrm[:], vsrc[:, f, :], ln_t[:, f:f + 1],
                            rstd_b[:],
                            mybir.AluOpType.mult, mybir.AluOpType.mult)
                        if proj == 0:
                            nc.sync.dma_start(qa_sh_r[:, f, :], nrm[:])
                        else:
                            nc.sync.dma_start(
                                ckv_sh[f * 128:(f + 1) * 128, :], nrm[:])
                # k_pe rope (local tokens) -> ckv_sh rows 512..576
                kpe = vap.tile([64, T_TILE], F32R, tag="kpe")
                nc.vector.tensor_copy(kpe[:], val_kv[0:64, 4, :])
                rps = pap.tile([64, T_TILE], F32, tag="rotk")
                nc.tensor.matmul(rps[:], p128_t[0:64, 0:64], kpe[:],
                                 start=True, stop=True)
                tmp = vap.tile([64, T_TILE], F32, tag="tmpk")
                nc.vector.tensor_tensor(tmp[:], cos_l[0:64, :], kpe[:],
                                        mybir.AluOpType.mult)
                rot = vap.tile([64, T_TILE], F32, tag="rotk2")
                nc.vector.tensor_tensor(rot[:], sin_l[0:64, :], rps[:],
                                        mybir.AluOpType.mult)
                kro = vap.tile([64, T_TILE], F32R, tag="kro")
                nc.vector.tensor_tensor(kro[:], tmp[:], rot[:],
                                        mybir.AluOpType.add)
                nc.sync.dma_start(ckv_sh[KVLR:KVLR + DR, :], kro[:])

                # gather: kv first (small, unblocks kv_b), then q_a
                nc.gpsimd.collective_compute(
                    "AllGather", mybir.AluOpType.bypass,
                    replica_groups=REPLICA_GROUPS,
                    ins=[ckv_sh[:]], outs=[ckv_all[:]])
                nc.gpsimd.collective_compute(
                    "AllGather", mybir.AluOpType.bypass,
                    replica_groups=REPLICA_GROUPS,
                    ins=[qa_sh[:]], outs=[qa_all[:]])

            # ---------- phase R: full rope cos/sin tables (for q) ----------
            cos_t = constp.tile([128, NT, T_TILE], F32)
            sin_t = constp.tile([128, NT, T_TILE], F32)
            with nc.named_scope("rope_tables"), \
                 tc.tile_pool(name="ropep", bufs=1) as rp:
                rope_tables(pos, S, cos_t.rearrange("p n t -> p (n t)"),
                            sin_t.rearrange("p n t -> p (n t)"), rp, "full")
                if debug:
                    nc.sync.dma_start(dbg_sin[:],
                                      sin_t.rearrange("p n t -> p (n t)"))
                    nc.sync.dma_start(dbg_cos[:],
                                      cos_t.rearrange("p n t -> p (n t)"))

            # C-phase resident tiles, direct-written by phase B
            kv_res = tc.alloc_tile_pool(name="kv_res", bufs=1)
            kn_sb = kv_res.tile([128, HPC, S], F32R, name="kn_sb")
            vh_sb = kv_res.tile([128, HPC, KB, DV], F32R, name="vh_sb")
            krot_sb = kv_res.tile([64, S], F32R, name="krot_sb")

            # ---------- phase B: q_b / kv_b / q rope ----------
            qT_nope_r = qT_nope.rearrange("(f p) s -> p f s", p=128)
            qT_rope_r2 = qT_rope.rearrange("(f p) s -> p f s", p=128)
            w_qbT_r = w_qbT.rearrange("(fi p) f -> p fi f", p=128)
            w_kvb_nT_r = w_kvb_nT.rearrange("(fi p) f -> p fi f", p=128)
            w_kvb_vT_r = w_kvb_vT.rearrange("(fi p) f -> p fi f", p=128)

            NQB = HPC * QD // 128  # 6 output tiles (4 nope + 2 rope-pair)

            with nc.named_scope("proj_b"), \
                 tc.tile_pool(name="wb", bufs=2) as wbp, \
                 tc.tile_pool(name="rhb", bufs=2) as rhbp, \
                 tc.tile_pool(name="evb", bufs=2) as evbp, \
                 tc.tile_pool(name="pb", bufs=2, space="PSUM") as pbp:
                wv_t = wbp.tile([128, NFO_KV, HPC * DV], F32R, name="wv_t",
                                bufs=1)
                nc.gpsimd.dma_start(wv_t[:], w_kvb_vT_r[:])
                for t in range(NT):
                    tsl = slice(t * T_TILE, (t + 1) * T_TILE)
                    qa_rhs = rhbp.tile([128, NFO_QA, T_TILE], F32R,
                                       tag="qarhs")
                    ckv_rhs = rhbp.tile([128, NFO_KV, T_TILE], F32R,
                                        tag="ckvrhs", bufs=1)
                    for f in range(NFO_QA):
                        nc.gpsimd.dma_start(
                            qa_rhs[:, f, :],
                            qa_all[t, f * 128:(f + 1) * 128, :])
                    for f in range(NFO_KV):
                        nc.gpsimd.dma_start(
                            ckv_rhs[:, f, :],
                            ckv_all[t, f * 128:(f + 1) * 128, :])
                    # k_rot arrives via the ckv AllGather
                    nc.gpsimd.dma_start(
                        krot_sb[:, tsl], ckv_all[t, KVLR:KVLR + DR, :])

                    # q_b: 4 nope tiles then 2 rope-pair tiles
                    for fo in range(NQB):
                        wt = wbp.tile([128, NFO_QA, 128], F32R, tag="wqb")
                        nc.gpsimd.dma_start(
                            wt[:], w_qbT_r[:, :, fo * 128:(fo + 1) * 128])
                        ps = pbp.tile([128, T_TILE], F32, tag="qb")
                        for fi in range(NFO_QA):
                            nc.tensor.matmul(ps[:], wt[:, fi, :],
                                             qa_rhs[:, fi, :],
                                             start=(fi == 0),
                                             stop=(fi == NFO_QA - 1))
                        if fo < HPC:  # nope
                            ev = evbp.tile([128, T_TILE], F32R, tag="evr")
                            nc.scalar.activation(
                                ev[:], ps[:],
                                mybir.ActivationFunctionType.Copy)
                            nc.sync.dma_start(qT_nope_r[:, fo, tsl], ev[:])
                        else:  # rope pair: rows = heads (2j, 2j+1)
                            qpe = evbp.tile([128, T_TILE], F32R, tag="evr")
                            nc.scalar.activation(
                                qpe[:], ps[:],
                                mybir.ActivationFunctionType.Copy)
                            rps = pbp.tile([128, T_TILE], F32, tag="rot",
                                           bufs=1)
                            nc.tensor.matmul(rps[:], p128_t[:], qpe[:],
                                             start=True, stop=True)
                            tmp = evbp.tile([128, T_TILE], F32, tag="tmp")
                            nc.vector.tensor_tensor(
                                tmp[:], cos_t[:, t, :], qpe[:],
                                mybir.AluOpType.mult)
                            rot = evbp.tile([128, T_TILE], F32, tag="rot2")
                            nc.vector.tensor_tensor(
                                rot[:], sin_t[:, t, :], rps[:],
                                mybir.AluOpType.mult)
                            qro = evbp.tile([128, T_TILE], F32R, tag="evr2")
                            nc.vector.tensor_tensor(
                                qro[:], tmp[:], rot[:], mybir.AluOpType.add)
                            j = fo - HPC
                            nc.sync.dma_start(qT_rope_r2[:, j, tsl], qro[:])

                    # kv_b nope -> straight into C-resident kn_sb
                    for fo in range(HPC):
                        wt = wbp.tile([128, NFO_KV, 128], F32R, tag="wkn")
                        nc.gpsimd.dma_start(
                            wt[:], w_kvb_nT_r[:, :, fo * 128:(fo + 1) * 128])
                        ps = pbp.tile([128, T_TILE], F32, tag="qb")
                        for fi in range(NFO_KV):
                            nc.tensor.matmul(ps[:], wt[:, fi, :],
                                             ckv_rhs[:, fi, :],
                                             start=(fi == 0),
                                             stop=(fi == NFO_KV - 1))
                        nc.scalar.activation(
                            kn_sb[:, fo, tsl], ps[:],
                            mybir.ActivationFunctionType.Copy)

                    # v (un-transposed) -> straight into C-resident vh_sb
                    for ts in range(T_TILE // 128):
                        kb = t * 4 + ts
                        ps = pbp.tile([128, HPC * DV], F32, tag="vps",
                                      bufs=2)
                        for fi in range(NFO_KV):
                            nc.tensor.matmul(
                                ps[:],
                                ckv_rhs[:, fi, ts * 128:(ts + 1) * 128],
                                wv_t[:, fi, :],
                                start=(fi == 0), stop=(fi == NFO_KV - 1))
                        nc.scalar.activation(
                            vh_sb[:, :, kb, :],
                            ps[:].rearrange("p (h d) -> p h d", h=HPC),
                            mybir.ActivationFunctionType.Copy)

            # ---------- phase C: attention + fused o_proj ----------
            w_oT_r = w_oT.rearrange("(fs p) hid -> p fs hid", p=128)
            qT_rope_r = qT_rope.rearrange("(f p) s -> p f s", p=64)
            with nc.named_scope("attn"), \
                 tc.tile_pool(name="cw", bufs=1) as cwp, \
                 tc.tile_pool(name="qrh", bufs=3) as qrhp, \
                 tc.tile_pool(name="pt", bufs=4) as ptp, \
                 tc.tile_pool(name="ao", bufs=2) as aop, \
                 tc.tile_pool(name="oe", bufs=3) as oep, \
                 tc.tile_pool(name="sps", bufs=2, space="PSUM") as spsp, \
                 tc.tile_pool(name="avs", bufs=2, space="PSUM") as avsp, \
                 tc.tile_pool(name="lps", bufs=2, space="PSUM") as lpsp, \
                 tc.tile_pool(name="pos_", bufs=2, space="PSUM") as posp:
                masks_t = cwp.tile([128, 4, T_TILE], F32R, name="masks_t")
                nc.sync.dma_start(masks_t[:], masks.rearrange("j p t -> p j t"))
                wo_sb = cwp.tile([128, HPC, HID], F32R, name="wo_sb")
                nc.sync.dma_start(wo_sb[:], w_oT_r[:])
                if debug:
                    nc.sync.dma_start(
                        dbg_kn.rearrange("(f p) s -> p f s", p=128)[:],
                        kn_sb[:])
                    nc.sync.dma_start(
                        dbg_v.rearrange("kb p h d -> p h kb d"), vh_sb[:])
                    nc.sync.dma_start(dbg_krot[:], krot_sb[:])
                for qt in range(NT):
                    qsl = slice(qt * T_TILE, (qt + 1) * T_TILE)
                    at_full = aop.tile([128, HPC, T_TILE], F32R, tag="atf")
                    nkb = 4 * qt + 4
                    for h in range(HPC):
                        qn_rhs = qrhp.tile([128, T_TILE], F32R, tag="qn")
                        nc.sync.dma_start(qn_rhs[:], qT_nope_r[:, h, qsl])
                        qr_rhs = qrhp.tile([64, T_TILE], F32R, tag="qr")
                        nc.sync.dma_start(qr_rhs[:], qT_rope_r[:, h, qsl])
                        av_ps = avsp.tile([128, T_TILE], F32, tag="av")
                        l_ps = lpsp.tile([1, T_TILE], F32, tag="l")
                        for kb in range(nkb):
                            sps = spsp.tile([128, T_TILE], F32, tag="s")
                            nc.tensor.matmul(
                                sps[:],
                                kn_sb[:, h, kb * 128:(kb + 1) * 128],
                                qn_rhs[:], start=True, stop=False)
                            nc.tensor.matmul(
                                sps[:], krot_sb[:, kb * 128:(kb + 1) * 128],
                                qr_rhs[:], start=False, stop=True)
                            pt = ptp.tile([128, T_TILE], F32R, tag="p")
                            nc.scalar.activation(
                                pt[:], sps[:],
                                mybir.ActivationFunctionType.Exp, scale=SCALE)
                            j = kb - 4 * qt
                            if j >= 0:
                                nc.vector.tensor_tensor(
                                    pt[:], pt[:], masks_t[:, j, :],
                                    mybir.AluOpType.mult)
                            nc.tensor.matmul(
                                av_ps[:], vh_sb[:, h, kb, :], pt[:],
                                start=(kb == 0), stop=(kb == nkb - 1))
                            nc.tensor.matmul(
                                l_ps[:], ones_t[:], pt[:],
                                start=(kb == 0), stop=(kb == nkb - 1))
                        rec = qrhp.tile([1, T_TILE], F32, tag="rec")
                        nc.vector.reciprocal(rec[:], l_ps[:])
                        rec_b = qrhp.tile([128, T_TILE], F32, tag="recb")
                        nc.gpsimd.partition_broadcast(rec_b[:], rec[:])
                        nc.vector.tensor_tensor(
                            at_full[:, h, :], av_ps[:], rec_b[:],
                            mybir.AluOpType.mult)
                        if debug:
                            nc.sync.dma_start(attn_T.rearrange(
                                "(f p) s -> p f s", p=128)[:, h, qsl],
                                at_full[:, h, :])
                    # fused o_proj for this q-tile
                    for ts in range(T_TILE // 128):
                        tok0 = qt * T_TILE + ts * 128
                        for ho in range(HID // T_TILE):
                            ps = posp.tile([128, T_TILE], F32, tag="po")
                            for fs in range(HPC):
                                nc.tensor.matmul(
                                    ps[:],
                                    at_full[:, fs, ts * 128:(ts + 1) * 128],
                                    wo_sb[:, fs,
                                          ho * T_TILE:(ho + 1) * T_TILE],
                                    start=(fs == 0), stop=(fs == HPC - 1))
                            oe = oep.tile([128, T_TILE], F32, tag="oe")
                            nc.scalar.activation(
                                oe[:], ps[:],
                                mybir.ActivationFunctionType.Copy)
                            nc.sync.dma_start(
                                out[tok0:tok0 + 128,
                                    ho * T_TILE:(ho + 1) * T_TILE],
                                oe[:])
            kv_res.release()

    nc.compile()
    _BUILD_CACHE[key] = nc
    return nc


def _host_consts():
    ivf = (1.0 / (ROPE_BASE ** (np.arange(0, DR, 2, dtype=np.float64) / DR)))
    ivf = ivf.astype(np.float32)                       # [32]
    inv_freq128 = np.tile(ivf, 4).reshape(128, 1)

    rot = np.zeros((DR, DR), np.float32)               # rot(x) = P @ x
    for d in range(32):
        rot[d, d + 32] = -1.0
        rot[d + 32, d] = 1.0
    rotT = rot.T
    p128 = np.zeros((128, 128), np.float32)
    p128[:64, :64] = rotT
    p128[64:, 64:] = rotT

    kk = np.arange(128)[None, :, None]                 # [1,128,1]
    jj = np.arange(4)[:, None, None]                   # [4,1,1]
    qq = np.arange(T_TILE)[None, None, :]              # [1,1,512]
    masks = ((jj * 128 + kk) <= qq).astype(np.float32)  # [4,128,512]

    return inv_freq128, p128, masks


LAST_RES = None


def kernel(_debug=False, **inputs):
    hidden_states = np.asarray(inputs["hidden_states"], np.float32)
    position_ids = np.asarray(inputs["position_ids"])
    W_qa = np.asarray(inputs["W_qa"], np.float32)
    b_qa = np.asarray(inputs["b_qa"], np.float32)
    w_qa_ln = np.asarray(inputs["w_qa_ln"], np.float32)
    W_qb = np.asarray(inputs["W_qb"], np.float32)
    W_kva = np.asarray(inputs["W_kva"], np.float32)
    b_kva = np.asarray(inputs["b_kva"], np.float32)
    w_kva_ln = np.asarray(inputs["w_kva_ln"], np.float32)
    W_kvb = np.asarray(inputs["W_kvb"], np.float32)
    W_o = np.asarray(inputs["W_o"], np.float32)

    nc = build_kernel(debug=_debug)

    inv_freq128, p128, masks = _host_consts()

    w_qaT = np.ascontiguousarray(W_qa.T)
    w_kvaT = np.ascontiguousarray(W_kva.T)
    W_qb_h = W_qb.reshape(NH, QD, QLR)
    W_kvb_h = W_kvb.reshape(NH, DN + DV, KVLR)
    b_qa_t = np.ascontiguousarray(b_qa.reshape(NFO_QA, 128).T)
    b_kva_p = np.zeros(640, np.float32)
    b_kva_p[:NKV] = b_kva
    b_kva_t = np.ascontiguousarray(b_kva_p.reshape(5, 128).T)
    ln_qa_t = np.ascontiguousarray(w_qa_ln.reshape(-1, 128).T)
    ln_kva_t = np.ascontiguousarray(w_kva_ln.reshape(-1, 128).T)
    ones_col = np.ones((128, 1), np.float32)

    in_maps = []
    for c in range(N_CORES):
        b = c // TPG
        g = c % TPG
        hs = list(range(g * HPC, (g + 1) * HPC))
        # q_b columns: nope blocks by head then rope blocks by head
        qb_nope = np.concatenate([W_qb_h[h, :DN, :] for h in hs], 0)
        qb_rope = np.concatenate([W_qb_h[h, DN:, :] for h in hs], 0)
        w_qbT = np.ascontiguousarray(np.concatenate([qb_nope, qb_rope], 0).T)
        w_kvb_nT = np.ascontiguousarray(
            np.concatenate([W_kvb_h[h, :DN, :] for h in hs], 0).T)
        w_kvb_vT = np.ascontiguousarray(
            np.concatenate([W_kvb_h[h, DN:, :] for h in hs], 0).T)
        w_oT = np.ascontiguousarray(
            W_o[:, g * HPC * DV:(g + 1) * HPC * DV].T)
        pos_b = position_ids[b].astype(np.int32)
        in_maps.append({
            "xTl": np.ascontiguousarray(
                hidden_states[b].T[:, g * T_TILE:(g + 1) * T_TILE]),
            "w_qaT": w_qaT, "w_kvaT": w_kvaT,
            "w_qbT": w_qbT, "w_kvb_nT": w_kvb_nT, "w_kvb_vT": w_kvb_vT,
            "w_oT": w_oT,
            "b_qa": b_qa_t, "b_kva": b_kva_t,
            "ln_qa": ln_qa_t, "ln_kva": ln_kva_t,
            "pos": np.ascontiguousarray(pos_b.reshape(1, S)),
            "pos_l": np.ascontiguousarray(
                pos_b[g * T_TILE:(g + 1) * T_TILE].reshape(1, T_TILE)),
            "inv_freq": inv_freq128,
            "p128": p128, "ones_col": ones_col,
            "masks": masks,
        })

    res = run_bass_kernel_spmd(nc, in_maps, list(range(N_CORES)))
    global LAST_RES
    LAST_RES = res

    out = np.zeros((B, S, HID), np.float32)
    for c in range(N_CORES):
        out[c // TPG] += res.results[c]["out"]
    return out


if __name__ == "__main__":
    import time
    t0 = time.time()
    build_kernel()
    print(f"build+compile: {time.time()-t0:.1f}s")
